# revision 24
# baseline (speedup 1.0000x reference)
"""Trainium2 Bass kernel for AlphaFold-style pair attention (nn_Attention_90211493085692).

Reference computation (per batch b=1):
    q = (q_x @ w_q.T) / sqrt(C)         -> [N, Q, H, C]
    k = kv_x @ w_k.T ; v = kv_x @ w_v.T
    a = softmax(q @ k.T + mask_bias + pair_bias)   (softmax over k)
    o = (a @ v) * sigmoid(q_x @ w_g.T)
    out = o @ w_o.T

Sharding: outer pair dim N=256 split across 8 cores (32 rows each);
weights / pair_bias replicated, each core computes its slab independently.

Device-side strategy (per core, ROWS = 32*256 = 8192 token rows):
  - host pre-transposes q_x/kv_x slabs to [CQ=128, ROWS] bf16 so matmul
    contraction dims sit on SBUF partitions.
  - projections q^/k^ in transposed layout [hc, rows]; gate via tanh on ACT
    (sigmoid(x) = 0.5*(1+tanh(x/2)), the 0.5 baked into the softmax sums);
    v in natural layout [rows, hc].
  - scores per row n: scoresT[k, (h,q)] via 4-head row-packed K=32 matmuls.
    pair_bias handling is split to balance engines:
      t=0 (k in [0,128)):  identity-matmul copies pbT into PSUM first, score
                           matmuls accumulate on top -> exp(s+pb) directly.
      t=1 (k in [128,256)): exp(s) then one bf16 2x-mode DVE multiply by
                           precomputed exp(pb).
  - sums over k via col-packed ones(=2.0) matmul; attn@V col-packed with
    v-natural stationary -> o[hc, q]; gate+normalize: go = ((1+tanh)*o)*inv
    (STT from PSUM, then 2x TT); W_O with w_o^T stationary -> outT[cq, q];
    DVE copy PSUM->SBUF bf16 batched per 2 rows; DMA out; host transposes.
"""

import sys

sys.path.insert(0, "/opt/trn_rl_repo")
sys.path.insert(0, "/opt/pypackages")

from contextlib import ExitStack

import ml_dtypes
import numpy as np

import concourse.bass as bass
import concourse.bacc as bacc
import concourse.tile as tile
from concourse import mybir

H = 4
C = 32
CQ = 128
N = 256
B = 1
NCORES = 8
NLOC = N // NCORES          # 32 outer rows per core
ROWS = NLOC * N             # 8192 token rows per core
P = 128

F32 = mybir.dt.float32
BF16 = mybir.dt.bfloat16
NP_BF16 = ml_dtypes.bfloat16
AF = mybir.ActivationFunctionType


# pair_bias application for the k∈[0,128) half: True = identity-matmul into
# PSUM (PE), False = exp(pb) multiply on DVE (like the k∈[128,256) half).
IDENT_PB = False


def build_nc(use_mask: bool = False) -> bass.Bass:
    nc = bacc.Bacc()

    q_xt = nc.declare_dram_parameter("q_xt", [CQ, ROWS], BF16, isOutput=False)
    kv_xt = nc.declare_dram_parameter("kv_xt", [CQ, ROWS], BF16, isOutput=False)
    pbias = nc.declare_dram_parameter("pbias", [2, P, H, N], BF16, isOutput=False)
    ident = nc.declare_dram_parameter("ident", [P, P], BF16, isOutput=False)
    maskt = nc.declare_dram_parameter("maskt", [2, P, NLOC], F32, isOutput=False)
    w_qt = nc.declare_dram_parameter("w_qt", [CQ, P], BF16, isOutput=False)
    w_kt = nc.declare_dram_parameter("w_kt", [CQ, P], BF16, isOutput=False)
    w_vt = nc.declare_dram_parameter("w_vt", [CQ, P], BF16, isOutput=False)
    w_gt = nc.declare_dram_parameter("w_gt", [CQ, P], BF16, isOutput=False)
    w_ot = nc.declare_dram_parameter("w_ot", [P, CQ], BF16, isOutput=False)
    outT = nc.declare_dram_parameter("outT", [CQ, ROWS], BF16, isOutput=True)

    NCHUNK = 4
    CROWS = ROWS // NCHUNK         # 2048 rows per chunk
    NPC = NLOC // NCHUNK           # 8 attention rows per chunk

    with tile.TileContext(nc) as tc, ExitStack() as ctx:
        const = ctx.enter_context(tc.tile_pool(name="const", bufs=1))

        qx_sb = [const.tile([P, CROWS], BF16, name=f"qx_sb{c}")
                 for c in range(NCHUNK)]
        kx_sb = [const.tile([P, CROWS], BF16, name=f"kx_sb{c}")
                 for c in range(NCHUNK)]
        for c in range(NCHUNK):
            # alternate HWDGE issue engines so the two big loads stream in
            # parallel
            nc.sync.dma_start(out=qx_sb[c][:], in_=q_xt[:, c * CROWS:(c + 1) * CROWS])
            nc.scalar.dma_start(out=kx_sb[c][:], in_=kv_xt[:, c * CROWS:(c + 1) * CROWS])

        wq_sb = const.tile([CQ, P], BF16)
        nc.sync.dma_start(out=wq_sb[:], in_=w_qt[:])
        wk_sb = const.tile([CQ, P], BF16)
        nc.sync.dma_start(out=wk_sb[:], in_=w_kt[:])
        wv_sb = const.tile([CQ, P], BF16)
        nc.sync.dma_start(out=wv_sb[:], in_=w_vt[:])
        wg_sb = const.tile([CQ, P], BF16)
        nc.sync.dma_start(out=wg_sb[:], in_=w_gt[:])
        wo_sb = const.tile([P, CQ], BF16)
        nc.sync.dma_start(out=wo_sb[:], in_=w_ot[:])
        if IDENT_PB:
            id_sb = const.tile([P, P], BF16)
            nc.scalar.dma_start(out=id_sb[:], in_=ident[:])

        mask_sb = const.tile([P, 2, NLOC], F32)
        nc.sync.dma_start(out=mask_sb[:], in_=maskt.rearrange("t p n -> p t n"))

        # pbT[p, t, h, q] = pair_bias[h, q, k = t*128+p]
        pb_sb = const.tile([P, 2, H, N], BF16)
        nc.scalar.dma_start(out=pb_sb[:], in_=pbias.rearrange("t p h q -> p t h q"))
        expb_sb = const.tile([P, 2, H, N], BF16)
        if IDENT_PB:
            nc.scalar.activation(out=expb_sb[:, 1], in_=pb_sb[:, 1], func=AF.Exp)
        else:
            nc.scalar.activation(out=expb_sb[:], in_=pb_sb[:], func=AF.Exp)

        # 2.0 (not 1.0): bakes the 0.5 of sigmoid(x)=0.5*(1+tanh(x/2)) into
        # the softmax denominator so inv = 0.5/sum
        ones_sb = const.tile([P, C], BF16)
        nc.vector.memset(ones_sb[:], 2.0)

        qhat = const.tile([P, ROWS], BF16)
        khat = const.tile([P, ROWS], BF16)
        ghat = const.tile([P, ROWS], BF16)
        vnat = const.tile([P, ROWS // P, P], BF16)

        PCH = 512                       # proj psum chunk (1 bank)
        with tc.tile_pool(name="scps", bufs=2, space="PSUM") as sp, \
             tc.tile_pool(name="accps", bufs=4, space="PSUM") as acc, \
             tc.tile_pool(name="work", bufs=2) as wrk, \
             tc.tile_pool(name="expp", bufs=3) as expp:

            def emit_proj_half(c, ch, half):
                """Projections for half of a 512-row slice of chunk c: PE
                filler emitted inside attention rows so the tensor engine
                has work while ACT runs the exps. half 0 = {q, g}, 1 = {k, v}."""
                base = c * CROWS
                sl = slice(ch * PCH, (ch + 1) * PCH)
                gsl = slice(base + ch * PCH, base + (ch + 1) * PCH)
                if half == 0:
                    ps_q = acc.tile([P, PCH], F32, tag="acc",
                                    name=f"ps_q{c}_{ch}")
                    nc.tensor.matmul(ps_q[:], lhsT=wq_sb[:],
                                     rhs=qx_sb[c][:, sl],
                                     start=True, stop=True)
                    nc.vector.tensor_copy(qhat[:, gsl], ps_q[:])

                    ps_g = acc.tile([P, PCH], F32, tag="acc",
                                    name=f"ps_g{c}_{ch}")
                    nc.tensor.matmul(ps_g[:], lhsT=wg_sb[:],
                                     rhs=qx_sb[c][:, sl],
                                     start=True, stop=True)
                    # tanh lives in the same ACT table set as exp, avoiding
                    # per-chunk table reloads; sigmoid(x)=0.5*(1+tanh(x/2))
                    nc.scalar.activation(out=ghat[:, gsl], in_=ps_g[:],
                                         func=AF.Tanh, scale=0.5)
                else:
                    ps_k = acc.tile([P, PCH], F32, tag="acc",
                                    name=f"ps_k{c}_{ch}")
                    nc.tensor.matmul(ps_k[:], lhsT=wk_sb[:],
                                     rhs=kx_sb[c][:, sl],
                                     start=True, stop=True)
                    nc.vector.tensor_copy(khat[:, gsl], ps_k[:])

                    ps_v = acc.tile([P, 4, P], F32, tag="acc",
                                    name=f"ps_v{c}_{ch}")
                    for j in range(4):
                        rt = ch * 4 + j
                        nc.tensor.matmul(
                            ps_v[:, j, :],
                            lhsT=kx_sb[c][:, rt * P:(rt + 1) * P],
                            rhs=wv_sb[:],
                            start=(j == 0), stop=(j == 3))
                    goff = base // P + ch * 4
                    nc.vector.tensor_copy(vnat[:, goff:goff + 4, :], ps_v[:])

            pending_out = []

            def flush_out():
                while pending_out:
                    psu, pq0 = pending_out.pop(0)
                    osb = wrk.tile([P, N], BF16, tag="osb", name=f"osb{pq0}")
                    nc.vector.tensor_copy(osb[:], psu[:, 1, :])
                    nc.sync.dma_start(out=outT[:, pq0:pq0 + N], in_=osb[:])

            def emit_attn_row(n, filler=None):
                q0 = n * N
                # drain the previous row's W_O output first: it is ready, so
                # it never blocks the DVE queue, and it frees su's bank early
                flush_out()

                # ---- scores: two 2-bank tiles (head-pair each: bank = h%2,
                # FD-halves = t) so 4 row-packed matmuls drain into 4 distinct
                # banks (same-bank concurrent drains are a fatal PSUM
                # collision) AND bufs=2 lets row n+1's scores overlap row n's
                # exp — the stall that was keeping the PE clock-gated cold.
                scps = []
                for i in range(2):            # head pair i: heads 2i, 2i+1
                    scp = sp.tile([P, 2, 512], F32, tag="sc",
                                  name=f"sc_{n}_{i}")
                    if IDENT_PB:
                        for hi in range(2):
                            nc.tensor.matmul(
                                scp[:, hi, 0:N],
                                lhsT=id_sb[:],
                                rhs=pb_sb[:, 0, 2 * i + hi, :],
                                start=True, stop=False,
                                skip_group_check=True)
                    for t in range(2):
                        for hi in range(2):
                            h = 2 * i + hi
                            nc.tensor.matmul(
                                scp[:, hi, t * N:(t + 1) * N],
                                lhsT=khat[32 * h:32 * h + 32,
                                          q0 + t * P:q0 + (t + 1) * P],
                                rhs=qhat[32 * h:32 * h + 32, q0:q0 + N],
                                start=not (IDENT_PB and t == 0), stop=True,
                                tile_position=(32 * h, 0),
                                skip_group_check=True)
                    scps.append(scp)

                # exp per tile (FD=1024 contiguous); expPair[i][:, hi, t, :]
                expPair = []
                for i in range(2):
                    er = expp.tile([P, 2, 2, N], BF16, tag=f"expR{i}",
                                   name=f"expR{i}_{n}")
                    if use_mask:
                        # mask bias differs per k-half: two strided calls
                        for t in range(2):
                            nc.scalar.activation(
                                out=er[:, :, t, :],
                                in_=scps[i][:, :, t * N:(t + 1) * N],
                                func=AF.Exp, bias=mask_sb[:, t, n:n + 1])
                    else:
                        nc.scalar.activation(out=er[:], in_=scps[i][:],
                                             func=AF.Exp)
                    expPair.append(er)
                expT1 = []
                for i in range(2):
                    em = expp.tile([P, 2, N], BF16, tag=f"expT{i}",
                                   name=f"expT{i}_{n}")
                    nc.vector.tensor_mul(out=em[:],
                                         in0=expPair[i][:, :, 1, :],
                                         in1=expb_sb[:, 1, 2 * i:2 * i + 2, :])
                    expT1.append(em)
                if not IDENT_PB:
                    expT0 = []
                    for i in range(2):
                        em = expp.tile([P, 2, N], BF16, tag=f"expS{i}",
                                       name=f"expS{i}_{n}")
                        nc.gpsimd.tensor_mul(
                            out=em[:], in0=expPair[i][:, :, 0, :],
                            in1=expb_sb[:, 0, 2 * i:2 * i + 2, :])
                        expT0.append(em)

                def rhs_for(t, h):
                    i, hi = h // 2, h % 2
                    if t == 1:
                        return expT1[i][:, hi, :]
                    if IDENT_PB:
                        return expPair[i][:, hi, 0, :]
                    return expT0[i][:, hi, :]

                # PE filler: next chunk's projection slice runs on the
                # tensor engine while ACT computes this row's exps.
                if filler is not None:
                    filler()

                # ---- sums (ones) + attn@V, col-packed, accum over t ----
                # separate banks: a start=True matmul clears has_written for
                # its whole bank row, so sums and o cannot share a bank.
                # su's second half is reused later for the W_O output (bank
                # bit-clears don't touch data outside the written region).
                su = acc.tile([P, 2, N], F32, tag="acc", name=f"su{n}")
                ov = acc.tile([P, 2, N], F32, tag="acc", name=f"ov{n}")
                for t in range(2):
                    for h in range(H):
                        nc.tensor.matmul(
                            su[32 * h:32 * h + 32, 0, :],
                            lhsT=ones_sb[:],
                            rhs=rhs_for(t, h),
                            start=(t == 0), stop=(t == 1),
                            tile_position=(0, 32 * h),
                            skip_group_check=True)
                    for h in range(H):
                        nc.tensor.matmul(
                            ov[32 * h:32 * h + 32, 0, :],
                            lhsT=vnat[:, q0 // P + t, 32 * h:32 * h + 32],
                            rhs=rhs_for(t, h),
                            start=(t == 0), stop=(t == 1),
                            tile_position=(0, 32 * h),
                            skip_group_check=True)

                # ---- normalize + gate:  go = ((1+tanh)*o) * (0.5/sum) ----
                inv = wrk.tile([P, N], F32, tag="inv", name=f"inv{n}")
                nc.vector.reciprocal_approx_fast(out=inv[:], in_=su[:, 0, :])
                go1 = wrk.tile([P, N], BF16, tag="go1", name=f"go1_{n}")
                nc.vector.scalar_tensor_tensor(
                    out=go1[:], in0=ghat[:, q0:q0 + N], scalar=1.0,
                    in1=ov[:, 0, :],
                    op0=mybir.AluOpType.add, op1=mybir.AluOpType.mult)
                go = wrk.tile([P, N], BF16, tag="go", name=f"go{n}")
                nc.gpsimd.tensor_mul(out=go[:], in0=go1[:], in1=inv[:])

                # ---- W_O (w_o^T stationary): outT[cq, q] into su[:, 1, :] ----
                nc.tensor.matmul(su[:, 1, :],
                                 lhsT=wo_sb[:], rhs=go[:],
                                 start=True, stop=True,
                                 skip_group_check=True)
                pending_out.append((su, q0))

            # chunk 0's projections run up front; chunk c+1's are spread as
            # PE filler across chunk c's attention rows (8 halves / 8 rows)
            NPIECE = CROWS // PCH          # 4 proj pieces per chunk
            for ch in range(NPIECE):
                emit_proj_half(0, ch, 0)
                emit_proj_half(0, ch, 1)
            for c in range(NCHUNK):
                for i, n in enumerate(range(c * NPC, (c + 1) * NPC)):
                    filler = None
                    if c + 1 < NCHUNK:
                        filler = (lambda cc, cch, hh: lambda:
                                  emit_proj_half(cc, cch, hh))(
                                      c + 1, i // 2, i % 2)
                    emit_attn_row(n, filler=filler)
            flush_out()

    nc.compile()
    return nc


_CACHE: dict = {}


def _get_nc(use_mask: bool = False) -> bass.Bass:
    key = ("nc", use_mask)
    if key not in _CACHE:
        _CACHE[key] = build_nc(use_mask=use_mask)
    return _CACHE[key]


def make_in_maps(q_x, kv_x, mask_bias, pair_bias, w_q, w_k, w_v, w_g, w_o):
    qf = np.asarray(q_x, dtype=np.float32).reshape(NCORES, ROWS, CQ)
    kf = np.asarray(kv_x, dtype=np.float32).reshape(NCORES, ROWS, CQ)
    mb = np.asarray(mask_bias, dtype=np.float32).reshape(N, N)      # [n, k]
    pb = np.asarray(pair_bias, dtype=np.float32).reshape(H, N, N)   # [h, q, k]

    pbT = np.transpose(pb, (2, 0, 1))                 # [k, h, q]
    pb_dev = np.ascontiguousarray(pbT.reshape(2, P, H, N)).astype(NP_BF16)
    ident = np.eye(P, dtype=NP_BF16)

    wqt = np.ascontiguousarray((w_q / np.sqrt(C)).T).astype(NP_BF16)
    wkt = np.ascontiguousarray(np.asarray(w_k).T).astype(NP_BF16)
    wvt = np.ascontiguousarray(np.asarray(w_v).T).astype(NP_BF16)
    wgt = np.ascontiguousarray(np.asarray(w_g).T).astype(NP_BF16)
    wot = np.ascontiguousarray(np.asarray(w_o).T).astype(NP_BF16)   # [hc, cq]

    in_maps = []
    for c in range(NCORES):
        m = mb[c * NLOC:(c + 1) * NLOC]               # [nloc, k]
        mT = np.ascontiguousarray(
            np.transpose(m.reshape(NLOC, 2, P), (1, 2, 0))).astype(np.float32)
        in_maps.append({
            "q_xt": np.ascontiguousarray(qf[c].T).astype(NP_BF16),
            "kv_xt": np.ascontiguousarray(kf[c].T).astype(NP_BF16),
            "pbias": pb_dev,
            "ident": ident,
            "maskt": mT,
            "w_qt": wqt, "w_kt": wkt, "w_vt": wvt, "w_gt": wgt, "w_ot": wot,
        })
    return in_maps


def gather_out(res) -> np.ndarray:
    outs = []
    for c in range(NCORES):
        ot = np.asarray(res.results[c]["outT"], dtype=np.float32)  # [CQ, ROWS]
        outs.append(ot.T)                                          # [ROWS, CQ]
    return np.concatenate(outs, axis=0).reshape(B, N, N, CQ)


def kernel(q_x, kv_x, mask_bias, pair_bias, w_q, w_k, w_v, w_g, w_o):
    from concourse.bass_utils import run_bass_kernel_spmd

    nc = _get_nc(use_mask=bool(np.any(np.asarray(mask_bias))))
    in_maps = make_in_maps(q_x, kv_x, mask_bias, pair_bias,
                           w_q, w_k, w_v, w_g, w_o)
    res = run_bass_kernel_spmd(nc, in_maps, list(range(NCORES)))
    return gather_out(res)


# revision 27
# speedup vs baseline: 1.2252x; 1.2252x over previous
"""Trainium2 Bass kernel for AlphaFold-style pair attention (nn_Attention_90211493085692).

Reference computation (per batch b=1):
    q = (q_x @ w_q.T) / sqrt(C)         -> [N, Q, H, C]
    k = kv_x @ w_k.T ; v = kv_x @ w_v.T
    a = softmax(q @ k.T + mask_bias + pair_bias)   (softmax over k)
    o = (a @ v) * sigmoid(q_x @ w_g.T)
    out = o @ w_o.T

Sharding: outer pair dim N=256 split across 8 cores (32 rows each);
weights / pair_bias replicated, each core computes its slab independently.

Device-side strategy (per core, ROWS = 32*256 = 8192 token rows):
  - host pre-transposes q_x/kv_x slabs to [CQ=128, ROWS] bf16 so matmul
    contraction dims sit on SBUF partitions.
  - projections q^/k^ in transposed layout [hc, rows]; gate via tanh on ACT
    (sigmoid(x) = 0.5*(1+tanh(x/2)), the 0.5 baked into the softmax sums);
    v in natural layout [rows, hc].
  - scores per row n: scoresT[k, (h,q)] via 4-head row-packed K=32 matmuls.
    pair_bias handling is split to balance engines:
      t=0 (k in [0,128)):  identity-matmul copies pbT into PSUM first, score
                           matmuls accumulate on top -> exp(s+pb) directly.
      t=1 (k in [128,256)): exp(s) then one bf16 2x-mode DVE multiply by
                           precomputed exp(pb).
  - sums over k via col-packed ones(=2.0) matmul; attn@V col-packed with
    v-natural stationary -> o[hc, q]; gate+normalize: go = ((1+tanh)*o)*inv
    (STT from PSUM, then 2x TT); W_O with w_o^T stationary -> outT[cq, q];
    DVE copy PSUM->SBUF bf16 batched per 2 rows; DMA out; host transposes.
"""

import sys

sys.path.insert(0, "/opt/trn_rl_repo")
sys.path.insert(0, "/opt/pypackages")

from contextlib import ExitStack

import ml_dtypes
import numpy as np

import concourse.bass as bass
import concourse.bacc as bacc
import concourse.tile as tile
from concourse import mybir

H = 4
C = 32
CQ = 128
N = 256
B = 1
NCORES = 8
NLOC = N // NCORES          # 32 outer rows per core
ROWS = NLOC * N             # 8192 token rows per core
P = 128

F32 = mybir.dt.float32
BF16 = mybir.dt.bfloat16
NP_BF16 = ml_dtypes.bfloat16
AF = mybir.ActivationFunctionType


# pair_bias application for the k∈[0,128) half: True = identity-matmul into
# PSUM (PE), False = exp(pb) multiply on DVE (like the k∈[128,256) half).
IDENT_PB = True


def build_nc(use_mask: bool = False) -> bass.Bass:
    nc = bacc.Bacc()

    q_xt = nc.declare_dram_parameter("q_xt", [CQ, ROWS], BF16, isOutput=False)
    kv_xt = nc.declare_dram_parameter("kv_xt", [CQ, ROWS], BF16, isOutput=False)
    pbias = nc.declare_dram_parameter("pbias", [2, P, H, N], BF16, isOutput=False)
    ident = nc.declare_dram_parameter("ident", [P, P], BF16, isOutput=False)
    maskt = nc.declare_dram_parameter("maskt", [2, P, NLOC], F32, isOutput=False)
    w_qt = nc.declare_dram_parameter("w_qt", [CQ, P], BF16, isOutput=False)
    w_kt = nc.declare_dram_parameter("w_kt", [CQ, P], BF16, isOutput=False)
    w_vt = nc.declare_dram_parameter("w_vt", [CQ, P], BF16, isOutput=False)
    w_gt = nc.declare_dram_parameter("w_gt", [CQ, P], BF16, isOutput=False)
    w_ot = nc.declare_dram_parameter("w_ot", [P, CQ], BF16, isOutput=False)
    outT = nc.declare_dram_parameter("outT", [CQ, ROWS], BF16, isOutput=True)

    NCHUNK = 4
    CROWS = ROWS // NCHUNK         # 2048 rows per chunk
    NPC = NLOC // NCHUNK           # 8 attention rows per chunk

    with tile.TileContext(nc) as tc, ExitStack() as ctx:
        const = ctx.enter_context(tc.tile_pool(name="const", bufs=1))

        qx_sb = [const.tile([P, CROWS], BF16, name=f"qx_sb{c}")
                 for c in range(NCHUNK)]
        kx_sb = [const.tile([P, CROWS], BF16, name=f"kx_sb{c}")
                 for c in range(NCHUNK)]
        for c in range(NCHUNK):
            # alternate HWDGE issue engines so the two big loads stream in
            # parallel
            nc.sync.dma_start(out=qx_sb[c][:], in_=q_xt[:, c * CROWS:(c + 1) * CROWS])
            nc.scalar.dma_start(out=kx_sb[c][:], in_=kv_xt[:, c * CROWS:(c + 1) * CROWS])

        wq_sb = const.tile([CQ, P], BF16)
        nc.sync.dma_start(out=wq_sb[:], in_=w_qt[:])
        wk_sb = const.tile([CQ, P], BF16)
        nc.sync.dma_start(out=wk_sb[:], in_=w_kt[:])
        wv_sb = const.tile([CQ, P], BF16)
        nc.sync.dma_start(out=wv_sb[:], in_=w_vt[:])
        wg_sb = const.tile([CQ, P], BF16)
        nc.sync.dma_start(out=wg_sb[:], in_=w_gt[:])
        wo_sb = const.tile([P, CQ], BF16)
        nc.sync.dma_start(out=wo_sb[:], in_=w_ot[:])
        if IDENT_PB:
            id_sb = const.tile([P, P], BF16)
            nc.scalar.dma_start(out=id_sb[:], in_=ident[:])

        mask_sb = const.tile([P, 2, NLOC], F32)
        nc.sync.dma_start(out=mask_sb[:], in_=maskt.rearrange("t p n -> p t n"))

        # pbT[p, t, h, q] = pair_bias[h, q, k = t*128+p]
        pb_sb = const.tile([P, 2, H, N], BF16)
        nc.scalar.dma_start(out=pb_sb[:], in_=pbias.rearrange("t p h q -> p t h q"))
        expb_sb = const.tile([P, 2, H, N], BF16)
        if IDENT_PB:
            nc.scalar.activation(out=expb_sb[:, 1], in_=pb_sb[:, 1], func=AF.Exp)
        else:
            nc.scalar.activation(out=expb_sb[:], in_=pb_sb[:], func=AF.Exp)

        # 2.0 (not 1.0): bakes the 0.5 of sigmoid(x)=0.5*(1+tanh(x/2)) into
        # the softmax denominator so inv = 0.5/sum
        ones_sb = const.tile([P, C], BF16)
        nc.vector.memset(ones_sb[:], 2.0)

        qhat = const.tile([P, ROWS], BF16)
        khat = const.tile([P, ROWS], BF16)
        ghat = const.tile([P, ROWS], BF16)
        vnat = const.tile([P, ROWS // P, P], BF16)

        PCH = 512                       # proj psum chunk (1 bank)
        with tc.tile_pool(name="scps", bufs=2, space="PSUM") as sp, \
             tc.tile_pool(name="accps", bufs=4, space="PSUM") as acc, \
             tc.tile_pool(name="work", bufs=2) as wrk, \
             tc.tile_pool(name="expp", bufs=3) as expp:

            def emit_proj_half(c, ch, half):
                """Projections for half of a 512-row slice of chunk c: PE
                filler emitted inside attention rows so the tensor engine
                has work while ACT runs the exps. half 0 = {q, g}, 1 = {k, v}."""
                base = c * CROWS
                sl = slice(ch * PCH, (ch + 1) * PCH)
                gsl = slice(base + ch * PCH, base + (ch + 1) * PCH)
                if half == 0:
                    ps_q = acc.tile([P, PCH], F32, tag="acc",
                                    name=f"ps_q{c}_{ch}")
                    nc.tensor.matmul(ps_q[:], lhsT=wq_sb[:],
                                     rhs=qx_sb[c][:, sl],
                                     start=True, stop=True)
                    nc.vector.tensor_copy(qhat[:, gsl], ps_q[:])

                    ps_g = acc.tile([P, PCH], F32, tag="acc",
                                    name=f"ps_g{c}_{ch}")
                    nc.tensor.matmul(ps_g[:], lhsT=wg_sb[:],
                                     rhs=qx_sb[c][:, sl],
                                     start=True, stop=True)
                    # tanh lives in the same ACT table set as exp, avoiding
                    # per-chunk table reloads; sigmoid(x)=0.5*(1+tanh(x/2))
                    nc.scalar.activation(out=ghat[:, gsl], in_=ps_g[:],
                                         func=AF.Tanh, scale=0.5)
                else:
                    ps_k = acc.tile([P, PCH], F32, tag="acc",
                                    name=f"ps_k{c}_{ch}")
                    nc.tensor.matmul(ps_k[:], lhsT=wk_sb[:],
                                     rhs=kx_sb[c][:, sl],
                                     start=True, stop=True)
                    nc.vector.tensor_copy(khat[:, gsl], ps_k[:])

                    ps_v = acc.tile([P, 4, P], F32, tag="acc",
                                    name=f"ps_v{c}_{ch}")
                    for j in range(4):
                        rt = ch * 4 + j
                        nc.tensor.matmul(
                            ps_v[:, j, :],
                            lhsT=kx_sb[c][:, rt * P:(rt + 1) * P],
                            rhs=wv_sb[:],
                            start=(j == 0), stop=(j == 3))
                    goff = base // P + ch * 4
                    nc.vector.tensor_copy(vnat[:, goff:goff + 4, :], ps_v[:])

            pending_out = []
            wo2_box = [None]

            def flush_out():
                while pending_out:
                    pwo, pq0 = pending_out.pop(0)
                    osb = wrk.tile([P, 2, N], BF16, tag="osb", name=f"osb{pq0}")
                    # on ACT: the scalar engine has slack, the DVE doesn't
                    nc.scalar.copy(osb[:], pwo[:])
                    nc.sync.dma_start(out=outT[:, pq0:pq0 + 2 * N], in_=osb[:])

            def emit_attn_row(n, filler=None):
                q0 = n * N
                # drain the previous row's W_O output first: it is ready, so
                # it never blocks the DVE queue, and it frees su's bank early
                flush_out()

                # ---- scores: two 2-bank tiles (head-pair each: bank = h%2,
                # FD-halves = t) so 4 row-packed matmuls drain into 4 distinct
                # banks (same-bank concurrent drains are a fatal PSUM
                # collision) AND bufs=2 lets row n+1's scores overlap row n's
                # exp — the stall that was keeping the PE clock-gated cold.
                scps = []
                for i in range(2):            # head pair i: heads 2i, 2i+1
                    scp = sp.tile([P, 2, 512], F32, tag="sc",
                                  name=f"sc_{n}_{i}")
                    if IDENT_PB:
                        for hi in range(2):
                            nc.tensor.matmul(
                                scp[:, hi, 0:N],
                                lhsT=id_sb[:],
                                rhs=pb_sb[:, 0, 2 * i + hi, :],
                                start=True, stop=False,
                                skip_group_check=True)
                    for t in range(2):
                        for hi in range(2):
                            h = 2 * i + hi
                            nc.tensor.matmul(
                                scp[:, hi, t * N:(t + 1) * N],
                                lhsT=khat[32 * h:32 * h + 32,
                                          q0 + t * P:q0 + (t + 1) * P],
                                rhs=qhat[32 * h:32 * h + 32, q0:q0 + N],
                                start=not (IDENT_PB and t == 0), stop=True,
                                tile_position=(32 * h, 0),
                                skip_group_check=True)
                    scps.append(scp)

                # exp per tile (FD=1024 contiguous); expPair[i][:, hi, t, :]
                expPair = []
                for i in range(2):
                    er = expp.tile([P, 2, 2, N], BF16, tag=f"expR{i}",
                                   name=f"expR{i}_{n}")
                    if use_mask:
                        # mask bias differs per k-half: two strided calls
                        for t in range(2):
                            nc.scalar.activation(
                                out=er[:, :, t, :],
                                in_=scps[i][:, :, t * N:(t + 1) * N],
                                func=AF.Exp, bias=mask_sb[:, t, n:n + 1])
                    else:
                        nc.scalar.activation(out=er[:], in_=scps[i][:],
                                             func=AF.Exp)
                    expPair.append(er)
                expT1 = []
                for i in range(2):
                    em = expp.tile([P, 2, N], BF16, tag=f"expT{i}",
                                   name=f"expT{i}_{n}")
                    nc.vector.tensor_mul(out=em[:],
                                         in0=expPair[i][:, :, 1, :],
                                         in1=expb_sb[:, 1, 2 * i:2 * i + 2, :])
                    expT1.append(em)
                if not IDENT_PB:
                    expT0 = []
                    for i in range(2):
                        em = expp.tile([P, 2, N], BF16, tag=f"expS{i}",
                                       name=f"expS{i}_{n}")
                        nc.vector.tensor_mul(
                            out=em[:], in0=expPair[i][:, :, 0, :],
                            in1=expb_sb[:, 0, 2 * i:2 * i + 2, :])
                        expT0.append(em)

                def rhs_for(t, h):
                    i, hi = h // 2, h % 2
                    if t == 1:
                        return expT1[i][:, hi, :]
                    if IDENT_PB:
                        return expPair[i][:, hi, 0, :]
                    return expT0[i][:, hi, :]

                # PE filler: next chunk's projection slice runs on the
                # tensor engine while ACT computes this row's exps.
                if filler is not None:
                    filler()

                # ---- sums (ones) + attn@V, col-packed, accum over t ----
                # separate banks: a start=True matmul clears has_written for
                # its whole bank row, so sums and o cannot share a bank.
                # su's second half is reused later for the W_O output (bank
                # bit-clears don't touch data outside the written region).
                su = acc.tile([P, 2, N], F32, tag="acc", name=f"su{n}")
                ov = acc.tile([P, 2, N], F32, tag="acc", name=f"ov{n}")
                for t in range(2):
                    for h in range(H):
                        nc.tensor.matmul(
                            su[32 * h:32 * h + 32, 0, :],
                            lhsT=ones_sb[:],
                            rhs=rhs_for(t, h),
                            start=(t == 0), stop=(t == 1),
                            tile_position=(0, 32 * h),
                            skip_group_check=True)
                    for h in range(H):
                        nc.tensor.matmul(
                            ov[32 * h:32 * h + 32, 0, :],
                            lhsT=vnat[:, q0 // P + t, 32 * h:32 * h + 32],
                            rhs=rhs_for(t, h),
                            start=(t == 0), stop=(t == 1),
                            tile_position=(0, 32 * h),
                            skip_group_check=True)

                # ---- normalize + gate:  go = ((1+tanh)*o) * (0.5/sum) ----
                inv = wrk.tile([P, N], F32, tag="inv", name=f"inv{n}")
                nc.vector.reciprocal_approx_fast(out=inv[:], in_=su[:, 0, :])
                go1 = wrk.tile([P, N], BF16, tag="go1", name=f"go1_{n}")
                nc.vector.scalar_tensor_tensor(
                    out=go1[:], in0=ghat[:, q0:q0 + N], scalar=1.0,
                    in1=ov[:, 0, :],
                    op0=mybir.AluOpType.add, op1=mybir.AluOpType.mult)
                go = wrk.tile([P, N], BF16, tag="go", name=f"go{n}")
                nc.vector.tensor_mul(out=go[:], in0=go1[:], in1=inv[:])

                # ---- W_O (w_o^T stationary): outT[cq, q], row pairs share a
                # PSUM bank so osb copy + DMA go out once per 2 rows ----
                if n % 2 == 0:
                    wo2_box[0] = acc.tile([P, 2, N], F32, tag="acc",
                                          name=f"wo2_{n}")
                nc.tensor.matmul(wo2_box[0][:, n % 2, :],
                                 lhsT=wo_sb[:], rhs=go[:],
                                 start=True, stop=True,
                                 skip_group_check=True)
                if n % 2 == 1:
                    pending_out.append((wo2_box[0], q0 - N))

            # chunk 0's projections run up front; chunk c+1's are spread as
            # PE filler across chunk c's attention rows (8 halves / 8 rows)
            NPIECE = CROWS // PCH          # 4 proj pieces per chunk
            for ch in range(NPIECE):
                emit_proj_half(0, ch, 0)
                emit_proj_half(0, ch, 1)
            for c in range(NCHUNK):
                for i, n in enumerate(range(c * NPC, (c + 1) * NPC)):
                    filler = None
                    if c + 1 < NCHUNK:
                        filler = (lambda cc, cch, hh: lambda:
                                  emit_proj_half(cc, cch, hh))(
                                      c + 1, i // 2, i % 2)
                    emit_attn_row(n, filler=filler)
            flush_out()

    nc.compile()
    return nc


_CACHE: dict = {}


def _get_nc(use_mask: bool = False) -> bass.Bass:
    key = ("nc", use_mask)
    if key not in _CACHE:
        _CACHE[key] = build_nc(use_mask=use_mask)
    return _CACHE[key]


def make_in_maps(q_x, kv_x, mask_bias, pair_bias, w_q, w_k, w_v, w_g, w_o):
    qf = np.asarray(q_x, dtype=np.float32).reshape(NCORES, ROWS, CQ)
    kf = np.asarray(kv_x, dtype=np.float32).reshape(NCORES, ROWS, CQ)
    mb = np.asarray(mask_bias, dtype=np.float32).reshape(N, N)      # [n, k]
    pb = np.asarray(pair_bias, dtype=np.float32).reshape(H, N, N)   # [h, q, k]

    pbT = np.transpose(pb, (2, 0, 1))                 # [k, h, q]
    pb_dev = np.ascontiguousarray(pbT.reshape(2, P, H, N)).astype(NP_BF16)
    ident = np.eye(P, dtype=NP_BF16)

    wqt = np.ascontiguousarray((w_q / np.sqrt(C)).T).astype(NP_BF16)
    wkt = np.ascontiguousarray(np.asarray(w_k).T).astype(NP_BF16)
    wvt = np.ascontiguousarray(np.asarray(w_v).T).astype(NP_BF16)
    wgt = np.ascontiguousarray(np.asarray(w_g).T).astype(NP_BF16)
    wot = np.ascontiguousarray(np.asarray(w_o).T).astype(NP_BF16)   # [hc, cq]

    in_maps = []
    for c in range(NCORES):
        m = mb[c * NLOC:(c + 1) * NLOC]               # [nloc, k]
        mT = np.ascontiguousarray(
            np.transpose(m.reshape(NLOC, 2, P), (1, 2, 0))).astype(np.float32)
        in_maps.append({
            "q_xt": np.ascontiguousarray(qf[c].T).astype(NP_BF16),
            "kv_xt": np.ascontiguousarray(kf[c].T).astype(NP_BF16),
            "pbias": pb_dev,
            "ident": ident,
            "maskt": mT,
            "w_qt": wqt, "w_kt": wkt, "w_vt": wvt, "w_gt": wgt, "w_ot": wot,
        })
    return in_maps


def gather_out(res) -> np.ndarray:
    outs = []
    for c in range(NCORES):
        ot = np.asarray(res.results[c]["outT"], dtype=np.float32)  # [CQ, ROWS]
        outs.append(ot.T)                                          # [ROWS, CQ]
    return np.concatenate(outs, axis=0).reshape(B, N, N, CQ)


def kernel(q_x, kv_x, mask_bias, pair_bias, w_q, w_k, w_v, w_g, w_o):
    from concourse.bass_utils import run_bass_kernel_spmd

    nc = _get_nc(use_mask=bool(np.any(np.asarray(mask_bias))))
    in_maps = make_in_maps(q_x, kv_x, mask_bias, pair_bias,
                           w_q, w_k, w_v, w_g, w_o)
    res = run_bass_kernel_spmd(nc, in_maps, list(range(NCORES)))
    return gather_out(res)


# revision 28
# speedup vs baseline: 1.2640x; 1.0317x over previous
"""Trainium2 Bass kernel for AlphaFold-style pair attention (nn_Attention_90211493085692).

Reference computation (per batch b=1):
    q = (q_x @ w_q.T) / sqrt(C)         -> [N, Q, H, C]
    k = kv_x @ w_k.T ; v = kv_x @ w_v.T
    a = softmax(q @ k.T + mask_bias + pair_bias)   (softmax over k)
    o = (a @ v) * sigmoid(q_x @ w_g.T)
    out = o @ w_o.T

Sharding: outer pair dim N=256 split across 8 cores (32 rows each);
weights / pair_bias replicated, each core computes its slab independently.

Device-side strategy (per core, ROWS = 32*256 = 8192 token rows):
  - host pre-transposes q_x/kv_x slabs to [CQ=128, ROWS] bf16 so matmul
    contraction dims sit on SBUF partitions.
  - projections q^/k^ in transposed layout [hc, rows]; gate via tanh on ACT
    (sigmoid(x) = 0.5*(1+tanh(x/2)), the 0.5 baked into the softmax sums);
    v in natural layout [rows, hc].
  - scores per row n: scoresT[k, (h,q)] via 4-head row-packed K=32 matmuls.
    pair_bias handling is split to balance engines:
      t=0 (k in [0,128)):  identity-matmul copies pbT into PSUM first, score
                           matmuls accumulate on top -> exp(s+pb) directly.
      t=1 (k in [128,256)): exp(s) then one bf16 2x-mode DVE multiply by
                           precomputed exp(pb).
  - sums over k via col-packed ones(=2.0) matmul; attn@V col-packed with
    v-natural stationary -> o[hc, q]; gate+normalize: go = ((1+tanh)*o)*inv
    (STT from PSUM, then 2x TT); W_O with w_o^T stationary -> outT[cq, q];
    DVE copy PSUM->SBUF bf16 batched per 2 rows; DMA out; host transposes.
"""

import sys

sys.path.insert(0, "/opt/trn_rl_repo")
sys.path.insert(0, "/opt/pypackages")

from contextlib import ExitStack

import ml_dtypes
import numpy as np

import concourse.bass as bass
import concourse.bacc as bacc
import concourse.tile as tile
from concourse import mybir

H = 4
C = 32
CQ = 128
N = 256
B = 1
NCORES = 8
NLOC = N // NCORES          # 32 outer rows per core
ROWS = NLOC * N             # 8192 token rows per core
P = 128

F32 = mybir.dt.float32
BF16 = mybir.dt.bfloat16
NP_BF16 = ml_dtypes.bfloat16
AF = mybir.ActivationFunctionType


# pair_bias application for the k∈[0,128) half: True = identity-matmul into
# PSUM (PE), False = exp(pb) multiply on DVE (like the k∈[128,256) half).
IDENT_PB = True


def build_nc(use_mask: bool = False) -> bass.Bass:
    nc = bacc.Bacc()

    q_xt = nc.declare_dram_parameter("q_xt", [CQ, ROWS], BF16, isOutput=False)
    kv_xt = nc.declare_dram_parameter("kv_xt", [CQ, ROWS], BF16, isOutput=False)
    pbias = nc.declare_dram_parameter("pbias", [2, P, H, N], BF16, isOutput=False)
    ident = nc.declare_dram_parameter("ident", [P, P], BF16, isOutput=False)
    maskt = nc.declare_dram_parameter("maskt", [2, P, NLOC], F32, isOutput=False)
    w_qt = nc.declare_dram_parameter("w_qt", [CQ, P], BF16, isOutput=False)
    w_kt = nc.declare_dram_parameter("w_kt", [CQ, P], BF16, isOutput=False)
    w_vt = nc.declare_dram_parameter("w_vt", [CQ, P], BF16, isOutput=False)
    w_gt = nc.declare_dram_parameter("w_gt", [CQ, P], BF16, isOutput=False)
    w_ot = nc.declare_dram_parameter("w_ot", [P, CQ], BF16, isOutput=False)
    outT = nc.declare_dram_parameter("outT", [CQ, ROWS], BF16, isOutput=True)

    NCHUNK = 4
    CROWS = ROWS // NCHUNK         # 2048 rows per chunk
    NPC = NLOC // NCHUNK           # 8 attention rows per chunk

    with tile.TileContext(nc) as tc, ExitStack() as ctx:
        const = ctx.enter_context(tc.tile_pool(name="const", bufs=1))

        # small constants FIRST on both queues: the first projection matmul
        # needs the weights, so they must not queue behind 2MB of input
        wq_sb = const.tile([CQ, P], BF16)
        nc.sync.dma_start(out=wq_sb[:], in_=w_qt[:])
        wk_sb = const.tile([CQ, P], BF16)
        nc.sync.dma_start(out=wk_sb[:], in_=w_kt[:])
        wv_sb = const.tile([CQ, P], BF16)
        nc.sync.dma_start(out=wv_sb[:], in_=w_vt[:])
        wg_sb = const.tile([CQ, P], BF16)
        nc.sync.dma_start(out=wg_sb[:], in_=w_gt[:])
        wo_sb = const.tile([P, CQ], BF16)
        nc.sync.dma_start(out=wo_sb[:], in_=w_ot[:])
        if IDENT_PB:
            id_sb = const.tile([P, P], BF16)
            nc.scalar.dma_start(out=id_sb[:], in_=ident[:])

        mask_sb = const.tile([P, 2, NLOC], F32)
        nc.sync.dma_start(out=mask_sb[:], in_=maskt.rearrange("t p n -> p t n"))

        # pbT[p, t, h, q] = pair_bias[h, q, k = t*128+p]
        pb_sb = const.tile([P, 2, H, N], BF16)
        nc.scalar.dma_start(out=pb_sb[:], in_=pbias.rearrange("t p h q -> p t h q"))

        qx_sb = [const.tile([P, CROWS], BF16, name=f"qx_sb{c}")
                 for c in range(NCHUNK)]
        kx_sb = [const.tile([P, CROWS], BF16, name=f"kx_sb{c}")
                 for c in range(NCHUNK)]
        for c in range(NCHUNK):
            # alternate HWDGE issue engines so the two big loads stream in
            # parallel
            nc.sync.dma_start(out=qx_sb[c][:], in_=q_xt[:, c * CROWS:(c + 1) * CROWS])
            nc.scalar.dma_start(out=kx_sb[c][:], in_=kv_xt[:, c * CROWS:(c + 1) * CROWS])
        expb_sb = const.tile([P, 2, H, N], BF16)
        if IDENT_PB:
            nc.scalar.activation(out=expb_sb[:, 1], in_=pb_sb[:, 1], func=AF.Exp)
        else:
            nc.scalar.activation(out=expb_sb[:], in_=pb_sb[:], func=AF.Exp)

        # 2.0 (not 1.0): bakes the 0.5 of sigmoid(x)=0.5*(1+tanh(x/2)) into
        # the softmax denominator so inv = 0.5/sum
        ones_sb = const.tile([P, C], BF16)
        nc.vector.memset(ones_sb[:], 2.0)

        qhat = const.tile([P, ROWS], BF16)
        khat = const.tile([P, ROWS], BF16)
        ghat = const.tile([P, ROWS], BF16)
        vnat = const.tile([P, ROWS // P, P], BF16)

        PCH = 512                       # proj psum chunk (1 bank)
        with tc.tile_pool(name="scps", bufs=2, space="PSUM") as sp, \
             tc.tile_pool(name="accps", bufs=4, space="PSUM") as acc, \
             tc.tile_pool(name="work", bufs=2) as wrk, \
             tc.tile_pool(name="expp", bufs=3) as expp:

            def emit_proj_half(c, ch, half):
                """Projections for half of a 512-row slice of chunk c: PE
                filler emitted inside attention rows so the tensor engine
                has work while ACT runs the exps. half 0 = {q, g}, 1 = {k, v}."""
                base = c * CROWS
                sl = slice(ch * PCH, (ch + 1) * PCH)
                gsl = slice(base + ch * PCH, base + (ch + 1) * PCH)
                if half == 0:
                    ps_q = acc.tile([P, PCH], F32, tag="acc",
                                    name=f"ps_q{c}_{ch}")
                    nc.tensor.matmul(ps_q[:], lhsT=wq_sb[:],
                                     rhs=qx_sb[c][:, sl],
                                     start=True, stop=True)
                    nc.vector.tensor_copy(qhat[:, gsl], ps_q[:])

                    ps_g = acc.tile([P, PCH], F32, tag="acc",
                                    name=f"ps_g{c}_{ch}")
                    nc.tensor.matmul(ps_g[:], lhsT=wg_sb[:],
                                     rhs=qx_sb[c][:, sl],
                                     start=True, stop=True)
                    # tanh lives in the same ACT table set as exp, avoiding
                    # per-chunk table reloads; sigmoid(x)=0.5*(1+tanh(x/2))
                    nc.scalar.activation(out=ghat[:, gsl], in_=ps_g[:],
                                         func=AF.Tanh, scale=0.5)
                else:
                    ps_k = acc.tile([P, PCH], F32, tag="acc",
                                    name=f"ps_k{c}_{ch}")
                    nc.tensor.matmul(ps_k[:], lhsT=wk_sb[:],
                                     rhs=kx_sb[c][:, sl],
                                     start=True, stop=True)
                    nc.vector.tensor_copy(khat[:, gsl], ps_k[:])

                    ps_v = acc.tile([P, 4, P], F32, tag="acc",
                                    name=f"ps_v{c}_{ch}")
                    for j in range(4):
                        rt = ch * 4 + j
                        nc.tensor.matmul(
                            ps_v[:, j, :],
                            lhsT=kx_sb[c][:, rt * P:(rt + 1) * P],
                            rhs=wv_sb[:],
                            start=(j == 0), stop=(j == 3))
                    goff = base // P + ch * 4
                    nc.vector.tensor_copy(vnat[:, goff:goff + 4, :], ps_v[:])

            pending_out = []
            wo2_box = [None]

            def flush_out():
                while pending_out:
                    pwo, pq0 = pending_out.pop(0)
                    osb = wrk.tile([P, 2, N], BF16, tag="osb", name=f"osb{pq0}")
                    # on ACT: the scalar engine has slack, the DVE doesn't
                    nc.scalar.copy(osb[:], pwo[:])
                    nc.sync.dma_start(out=outT[:, pq0:pq0 + 2 * N], in_=osb[:])

            def emit_attn_row(n, filler=None):
                q0 = n * N
                # drain the previous row's W_O output first: it is ready, so
                # it never blocks the DVE queue, and it frees su's bank early
                flush_out()

                # ---- scores: two 2-bank tiles (head-pair each: bank = h%2,
                # FD-halves = t) so 4 row-packed matmuls drain into 4 distinct
                # banks (same-bank concurrent drains are a fatal PSUM
                # collision) AND bufs=2 lets row n+1's scores overlap row n's
                # exp — the stall that was keeping the PE clock-gated cold.
                scps = []
                for i in range(2):            # head pair i: heads 2i, 2i+1
                    scp = sp.tile([P, 2, 512], F32, tag="sc",
                                  name=f"sc_{n}_{i}")
                    if IDENT_PB:
                        for hi in range(2):
                            nc.tensor.matmul(
                                scp[:, hi, 0:N],
                                lhsT=id_sb[:],
                                rhs=pb_sb[:, 0, 2 * i + hi, :],
                                start=True, stop=False,
                                skip_group_check=True)
                    for t in range(2):
                        for hi in range(2):
                            h = 2 * i + hi
                            nc.tensor.matmul(
                                scp[:, hi, t * N:(t + 1) * N],
                                lhsT=khat[32 * h:32 * h + 32,
                                          q0 + t * P:q0 + (t + 1) * P],
                                rhs=qhat[32 * h:32 * h + 32, q0:q0 + N],
                                start=not (IDENT_PB and t == 0), stop=True,
                                tile_position=(32 * h, 0),
                                skip_group_check=True)
                    scps.append(scp)

                # exp per tile (FD=1024 contiguous); expPair[i][:, hi, t, :]
                expPair = []
                for i in range(2):
                    er = expp.tile([P, 2, 2, N], BF16, tag=f"expR{i}",
                                   name=f"expR{i}_{n}")
                    if use_mask:
                        # mask bias differs per k-half: two strided calls
                        for t in range(2):
                            nc.scalar.activation(
                                out=er[:, :, t, :],
                                in_=scps[i][:, :, t * N:(t + 1) * N],
                                func=AF.Exp, bias=mask_sb[:, t, n:n + 1])
                    else:
                        nc.scalar.activation(out=er[:], in_=scps[i][:],
                                             func=AF.Exp)
                    expPair.append(er)
                expT1 = []
                for i in range(2):
                    em = expp.tile([P, 2, N], BF16, tag=f"expT{i}",
                                   name=f"expT{i}_{n}")
                    nc.vector.tensor_mul(out=em[:],
                                         in0=expPair[i][:, :, 1, :],
                                         in1=expb_sb[:, 1, 2 * i:2 * i + 2, :])
                    expT1.append(em)
                if not IDENT_PB:
                    expT0 = []
                    for i in range(2):
                        em = expp.tile([P, 2, N], BF16, tag=f"expS{i}",
                                       name=f"expS{i}_{n}")
                        nc.vector.tensor_mul(
                            out=em[:], in0=expPair[i][:, :, 0, :],
                            in1=expb_sb[:, 0, 2 * i:2 * i + 2, :])
                        expT0.append(em)

                def rhs_for(t, h):
                    i, hi = h // 2, h % 2
                    if t == 1:
                        return expT1[i][:, hi, :]
                    if IDENT_PB:
                        return expPair[i][:, hi, 0, :]
                    return expT0[i][:, hi, :]

                # PE filler: next chunk's projection slice runs on the
                # tensor engine while ACT computes this row's exps.
                if filler is not None:
                    filler()

                # ---- sums (ones) + attn@V, col-packed, accum over t ----
                # separate banks: a start=True matmul clears has_written for
                # its whole bank row, so sums and o cannot share a bank.
                # su's second half is reused later for the W_O output (bank
                # bit-clears don't touch data outside the written region).
                su = acc.tile([P, 2, N], F32, tag="acc", name=f"su{n}")
                ov = acc.tile([P, 2, N], F32, tag="acc", name=f"ov{n}")
                for t in range(2):
                    for h in range(H):
                        nc.tensor.matmul(
                            su[32 * h:32 * h + 32, 0, :],
                            lhsT=ones_sb[:],
                            rhs=rhs_for(t, h),
                            start=(t == 0), stop=(t == 1),
                            tile_position=(0, 32 * h),
                            skip_group_check=True)
                    for h in range(H):
                        nc.tensor.matmul(
                            ov[32 * h:32 * h + 32, 0, :],
                            lhsT=vnat[:, q0 // P + t, 32 * h:32 * h + 32],
                            rhs=rhs_for(t, h),
                            start=(t == 0), stop=(t == 1),
                            tile_position=(0, 32 * h),
                            skip_group_check=True)

                # ---- normalize + gate:  go = ((1+tanh)*o) * (0.5/sum) ----
                inv = wrk.tile([P, N], F32, tag="inv", name=f"inv{n}")
                nc.vector.reciprocal_approx_fast(out=inv[:], in_=su[:, 0, :])
                go1 = wrk.tile([P, N], BF16, tag="go1", name=f"go1_{n}")
                nc.vector.scalar_tensor_tensor(
                    out=go1[:], in0=ghat[:, q0:q0 + N], scalar=1.0,
                    in1=ov[:, 0, :],
                    op0=mybir.AluOpType.add, op1=mybir.AluOpType.mult)
                go = wrk.tile([P, N], BF16, tag="go", name=f"go{n}")
                nc.vector.tensor_mul(out=go[:], in0=go1[:], in1=inv[:])

                # ---- W_O (w_o^T stationary): outT[cq, q], row pairs share a
                # PSUM bank so osb copy + DMA go out once per 2 rows ----
                if n % 2 == 0:
                    wo2_box[0] = acc.tile([P, 2, N], F32, tag="acc",
                                          name=f"wo2_{n}")
                nc.tensor.matmul(wo2_box[0][:, n % 2, :],
                                 lhsT=wo_sb[:], rhs=go[:],
                                 start=True, stop=True,
                                 skip_group_check=True)
                if n % 2 == 1:
                    pending_out.append((wo2_box[0], q0 - N))

            # chunk 0's projections run up front; chunk c+1's are spread as
            # PE filler across chunk c's attention rows (8 halves / 8 rows)
            NPIECE = CROWS // PCH          # 4 proj pieces per chunk
            for ch in range(NPIECE):
                emit_proj_half(0, ch, 0)
                emit_proj_half(0, ch, 1)
            for c in range(NCHUNK):
                for i, n in enumerate(range(c * NPC, (c + 1) * NPC)):
                    filler = None
                    if c + 1 < NCHUNK:
                        filler = (lambda cc, cch, hh: lambda:
                                  emit_proj_half(cc, cch, hh))(
                                      c + 1, i // 2, i % 2)
                    emit_attn_row(n, filler=filler)
            flush_out()

    nc.compile()
    return nc


_CACHE: dict = {}


def _get_nc(use_mask: bool = False) -> bass.Bass:
    key = ("nc", use_mask)
    if key not in _CACHE:
        _CACHE[key] = build_nc(use_mask=use_mask)
    return _CACHE[key]


def make_in_maps(q_x, kv_x, mask_bias, pair_bias, w_q, w_k, w_v, w_g, w_o):
    qf = np.asarray(q_x, dtype=np.float32).reshape(NCORES, ROWS, CQ)
    kf = np.asarray(kv_x, dtype=np.float32).reshape(NCORES, ROWS, CQ)
    mb = np.asarray(mask_bias, dtype=np.float32).reshape(N, N)      # [n, k]
    pb = np.asarray(pair_bias, dtype=np.float32).reshape(H, N, N)   # [h, q, k]

    pbT = np.transpose(pb, (2, 0, 1))                 # [k, h, q]
    pb_dev = np.ascontiguousarray(pbT.reshape(2, P, H, N)).astype(NP_BF16)
    ident = np.eye(P, dtype=NP_BF16)

    wqt = np.ascontiguousarray((w_q / np.sqrt(C)).T).astype(NP_BF16)
    wkt = np.ascontiguousarray(np.asarray(w_k).T).astype(NP_BF16)
    wvt = np.ascontiguousarray(np.asarray(w_v).T).astype(NP_BF16)
    wgt = np.ascontiguousarray(np.asarray(w_g).T).astype(NP_BF16)
    wot = np.ascontiguousarray(np.asarray(w_o).T).astype(NP_BF16)   # [hc, cq]

    in_maps = []
    for c in range(NCORES):
        m = mb[c * NLOC:(c + 1) * NLOC]               # [nloc, k]
        mT = np.ascontiguousarray(
            np.transpose(m.reshape(NLOC, 2, P), (1, 2, 0))).astype(np.float32)
        in_maps.append({
            "q_xt": np.ascontiguousarray(qf[c].T).astype(NP_BF16),
            "kv_xt": np.ascontiguousarray(kf[c].T).astype(NP_BF16),
            "pbias": pb_dev,
            "ident": ident,
            "maskt": mT,
            "w_qt": wqt, "w_kt": wkt, "w_vt": wvt, "w_gt": wgt, "w_ot": wot,
        })
    return in_maps


def gather_out(res) -> np.ndarray:
    outs = []
    for c in range(NCORES):
        ot = np.asarray(res.results[c]["outT"], dtype=np.float32)  # [CQ, ROWS]
        outs.append(ot.T)                                          # [ROWS, CQ]
    return np.concatenate(outs, axis=0).reshape(B, N, N, CQ)


def kernel(q_x, kv_x, mask_bias, pair_bias, w_q, w_k, w_v, w_g, w_o):
    from concourse.bass_utils import run_bass_kernel_spmd

    nc = _get_nc(use_mask=bool(np.any(np.asarray(mask_bias))))
    in_maps = make_in_maps(q_x, kv_x, mask_bias, pair_bias,
                           w_q, w_k, w_v, w_g, w_o)
    res = run_bass_kernel_spmd(nc, in_maps, list(range(NCORES)))
    return gather_out(res)


# revision 30
# speedup vs baseline: 1.3468x; 1.0655x over previous
"""Trainium2 Bass kernel for AlphaFold-style pair attention (nn_Attention_90211493085692).

Reference computation (per batch b=1):
    q = (q_x @ w_q.T) / sqrt(C)         -> [N, Q, H, C]
    k = kv_x @ w_k.T ; v = kv_x @ w_v.T
    a = softmax(q @ k.T + mask_bias + pair_bias)   (softmax over k)
    o = (a @ v) * sigmoid(q_x @ w_g.T)
    out = o @ w_o.T

Sharding: outer pair dim N=256 split across 8 cores (32 rows each);
weights / pair_bias replicated, each core computes its slab independently.

Device-side strategy (per core, ROWS = 32*256 = 8192 token rows):
  - host pre-transposes q_x/kv_x slabs to [CQ=128, ROWS] bf16 so matmul
    contraction dims sit on SBUF partitions.
  - projections q^/k^ in transposed layout [hc, rows]; gate via tanh on ACT
    (sigmoid(x) = 0.5*(1+tanh(x/2)), the 0.5 baked into the softmax sums);
    v in natural layout [rows, hc].
  - scores per row n: scoresT[k, (h,q)] via 4-head row-packed K=32 matmuls.
    pair_bias handling is split to balance engines:
      t=0 (k in [0,128)):  identity-matmul copies pbT into PSUM first, score
                           matmuls accumulate on top -> exp(s+pb) directly.
      t=1 (k in [128,256)): exp(s) then one bf16 2x-mode DVE multiply by
                           precomputed exp(pb).
  - sums over k via col-packed ones(=2.0) matmul; attn@V col-packed with
    v-natural stationary -> o[hc, q]; gate+normalize: go = ((1+tanh)*o)*inv
    (STT from PSUM, then 2x TT); W_O with w_o^T stationary -> outT[cq, q];
    DVE copy PSUM->SBUF bf16 batched per 2 rows; DMA out; host transposes.
"""

import sys

sys.path.insert(0, "/opt/trn_rl_repo")
sys.path.insert(0, "/opt/pypackages")

from contextlib import ExitStack

import ml_dtypes
import numpy as np

import concourse.bass as bass
import concourse.bacc as bacc
import concourse.tile as tile
from concourse import mybir

H = 4
C = 32
CQ = 128
N = 256
B = 1
NCORES = 8
NLOC = N // NCORES          # 32 outer rows per core
ROWS = NLOC * N             # 8192 token rows per core
P = 128

F32 = mybir.dt.float32
BF16 = mybir.dt.bfloat16
NP_BF16 = ml_dtypes.bfloat16
AF = mybir.ActivationFunctionType


# pair_bias application for the k∈[0,128) half: True = identity-matmul into
# PSUM (PE), False = exp(pb) multiply on DVE (like the k∈[128,256) half).
IDENT_PB = True


def build_nc(use_mask: bool = False) -> bass.Bass:
    nc = bacc.Bacc()

    q_xt = nc.declare_dram_parameter("q_xt", [CQ, ROWS], BF16, isOutput=False)
    kv_xt = nc.declare_dram_parameter("kv_xt", [CQ, ROWS], BF16, isOutput=False)
    pbias = nc.declare_dram_parameter("pbias", [2, P, H, N], BF16, isOutput=False)
    ident = nc.declare_dram_parameter("ident", [P, P], BF16, isOutput=False)
    maskt = nc.declare_dram_parameter("maskt", [2, P, NLOC], F32, isOutput=False)
    w_qt = nc.declare_dram_parameter("w_qt", [CQ, P], BF16, isOutput=False)
    w_kt = nc.declare_dram_parameter("w_kt", [CQ, P], BF16, isOutput=False)
    w_vt = nc.declare_dram_parameter("w_vt", [CQ, P], BF16, isOutput=False)
    w_gt = nc.declare_dram_parameter("w_gt", [CQ, P], BF16, isOutput=False)
    w_ot = nc.declare_dram_parameter("w_ot", [P, CQ], BF16, isOutput=False)
    outT = nc.declare_dram_parameter("outT", [CQ, ROWS], BF16, isOutput=True)

    NCHUNK = 4
    CROWS = ROWS // NCHUNK         # 2048 rows per chunk
    NPC = NLOC // NCHUNK           # 8 attention rows per chunk

    with tile.TileContext(nc) as tc, ExitStack() as ctx:
        const = ctx.enter_context(tc.tile_pool(name="const", bufs=1))

        # small constants FIRST on both queues: the first projection matmul
        # needs the weights, so they must not queue behind 2MB of input
        wq_sb = const.tile([CQ, P], BF16)
        nc.sync.dma_start(out=wq_sb[:], in_=w_qt[:])
        wk_sb = const.tile([CQ, P], BF16)
        nc.sync.dma_start(out=wk_sb[:], in_=w_kt[:])
        wv_sb = const.tile([CQ, P], BF16)
        nc.sync.dma_start(out=wv_sb[:], in_=w_vt[:])
        wg_sb = const.tile([CQ, P], BF16)
        nc.sync.dma_start(out=wg_sb[:], in_=w_gt[:])
        wo_sb = const.tile([P, CQ], BF16)
        nc.sync.dma_start(out=wo_sb[:], in_=w_ot[:])
        if IDENT_PB:
            id_sb = const.tile([P, P], BF16)
            nc.scalar.dma_start(out=id_sb[:], in_=ident[:])

        mask_sb = const.tile([P, 2, NLOC], F32)
        nc.sync.dma_start(out=mask_sb[:], in_=maskt.rearrange("t p n -> p t n"))

        # pbT[p, t, h, q] = pair_bias[h, q, k = t*128+p]
        pb_sb = const.tile([P, 2, H, N], BF16)
        nc.scalar.dma_start(out=pb_sb[:], in_=pbias.rearrange("t p h q -> p t h q"))

        qx_sb = [const.tile([P, CROWS], BF16, name=f"qx_sb{c}")
                 for c in range(NCHUNK)]
        kx_sb = [const.tile([P, CROWS], BF16, name=f"kx_sb{c}")
                 for c in range(NCHUNK)]
        for c in range(NCHUNK):
            # alternate HWDGE issue engines so the two big loads stream in
            # parallel
            nc.sync.dma_start(out=qx_sb[c][:], in_=q_xt[:, c * CROWS:(c + 1) * CROWS])
            nc.scalar.dma_start(out=kx_sb[c][:], in_=kv_xt[:, c * CROWS:(c + 1) * CROWS])
        expb_sb = const.tile([P, 2, H, N], BF16)
        if IDENT_PB:
            nc.scalar.activation(out=expb_sb[:, 1], in_=pb_sb[:, 1], func=AF.Exp)
        else:
            nc.scalar.activation(out=expb_sb[:], in_=pb_sb[:], func=AF.Exp)

        # 2.0 (not 1.0): bakes the 0.5 of sigmoid(x)=0.5*(1+tanh(x/2)) into
        # the softmax denominator so inv = 0.5/sum
        ones_sb = const.tile([P, C], BF16)
        nc.vector.memset(ones_sb[:], 2.0)

        qhat = const.tile([P, ROWS], BF16)
        khat = const.tile([P, ROWS], BF16)
        ghat = const.tile([P, ROWS], BF16)
        vnat = const.tile([P, ROWS // P, P], BF16)

        PCH = 512                       # proj psum chunk (1 bank)
        with tc.tile_pool(name="scps", bufs=2, space="PSUM") as sp, \
             tc.tile_pool(name="accps", bufs=4, space="PSUM") as acc, \
             tc.tile_pool(name="work", bufs=2) as wrk, \
             tc.tile_pool(name="expp", bufs=3) as expp:

            def emit_proj_half(c, ch, half):
                """Projections for half of a 512-row slice of chunk c: PE
                filler emitted inside attention rows so the tensor engine
                has work while ACT runs the exps. half 0 = {q, g}, 1 = {k, v}."""
                base = c * CROWS
                sl = slice(ch * PCH, (ch + 1) * PCH)
                gsl = slice(base + ch * PCH, base + (ch + 1) * PCH)
                if half == 0:
                    ps_q = acc.tile([P, PCH], F32, tag="acc",
                                    name=f"ps_q{c}_{ch}")
                    nc.tensor.matmul(ps_q[:], lhsT=wq_sb[:],
                                     rhs=qx_sb[c][:, sl],
                                     start=True, stop=True)
                    nc.vector.tensor_copy(qhat[:, gsl], ps_q[:])

                    ps_g = acc.tile([P, PCH], F32, tag="acc",
                                    name=f"ps_g{c}_{ch}")
                    nc.tensor.matmul(ps_g[:], lhsT=wg_sb[:],
                                     rhs=qx_sb[c][:, sl],
                                     start=True, stop=True)
                    # tanh lives in the same ACT table set as exp, avoiding
                    # per-chunk table reloads; sigmoid(x)=0.5*(1+tanh(x/2))
                    nc.scalar.activation(out=ghat[:, gsl], in_=ps_g[:],
                                         func=AF.Tanh, scale=0.5)
                else:
                    ps_k = acc.tile([P, PCH], F32, tag="acc",
                                    name=f"ps_k{c}_{ch}")
                    nc.tensor.matmul(ps_k[:], lhsT=wk_sb[:],
                                     rhs=kx_sb[c][:, sl],
                                     start=True, stop=True)
                    nc.vector.tensor_copy(khat[:, gsl], ps_k[:])

                    ps_v = acc.tile([P, 4, P], F32, tag="acc",
                                    name=f"ps_v{c}_{ch}")
                    for j in range(4):
                        rt = ch * 4 + j
                        nc.tensor.matmul(
                            ps_v[:, j, :],
                            lhsT=kx_sb[c][:, rt * P:(rt + 1) * P],
                            rhs=wv_sb[:],
                            start=(j == 0), stop=(j == 3))
                    goff = base // P + ch * 4
                    nc.vector.tensor_copy(vnat[:, goff:goff + 4, :], ps_v[:])

            pending_out = []

            def flush_out():
                while pending_out:
                    psu, pq0 = pending_out.pop(0)
                    osb = wrk.tile([P, N], BF16, tag="osb", name=f"osb{pq0}")
                    # on ACT: the scalar engine has slack, the DVE doesn't
                    nc.scalar.copy(osb[:], psu[:, 1, :])
                    nc.sync.dma_start(out=outT[:, pq0:pq0 + N], in_=osb[:])

            def emit_attn_row(n, filler=None):
                q0 = n * N
                # drain the previous row's W_O output first: it is ready, so
                # it never blocks the DVE queue, and it frees su's bank early
                flush_out()

                # ---- scores: two 2-bank tiles (head-pair each: bank = h%2,
                # FD-halves = t) so 4 row-packed matmuls drain into 4 distinct
                # banks (same-bank concurrent drains are a fatal PSUM
                # collision) AND bufs=2 lets row n+1's scores overlap row n's
                # exp — the stall that was keeping the PE clock-gated cold.
                scps = []
                for i in range(2):            # head pair i: heads 2i, 2i+1
                    scp = sp.tile([P, 2, 512], F32, tag="sc",
                                  name=f"sc_{n}_{i}")
                    if IDENT_PB:
                        for hi in range(2):
                            nc.tensor.matmul(
                                scp[:, hi, 0:N],
                                lhsT=id_sb[:],
                                rhs=pb_sb[:, 0, 2 * i + hi, :],
                                start=True, stop=False,
                                skip_group_check=True)
                    for t in range(2):
                        for hi in range(2):
                            h = 2 * i + hi
                            nc.tensor.matmul(
                                scp[:, hi, t * N:(t + 1) * N],
                                lhsT=khat[32 * h:32 * h + 32,
                                          q0 + t * P:q0 + (t + 1) * P],
                                rhs=qhat[32 * h:32 * h + 32, q0:q0 + N],
                                start=not (IDENT_PB and t == 0), stop=True,
                                tile_position=(32 * h, 0),
                                skip_group_check=True)
                    scps.append(scp)

                # exp per tile (FD=1024 contiguous); expPair[i][:, hi, t, :]
                expPair = []
                for i in range(2):
                    er = expp.tile([P, 2, 2, N], BF16, tag=f"expR{i}",
                                   name=f"expR{i}_{n}")
                    if use_mask:
                        # mask bias differs per k-half: two strided calls
                        for t in range(2):
                            nc.scalar.activation(
                                out=er[:, :, t, :],
                                in_=scps[i][:, :, t * N:(t + 1) * N],
                                func=AF.Exp, bias=mask_sb[:, t, n:n + 1])
                    else:
                        nc.scalar.activation(out=er[:], in_=scps[i][:],
                                             func=AF.Exp)
                    expPair.append(er)
                expT1 = []
                for i in range(2):
                    em = expp.tile([P, 2, N], BF16, tag=f"expT{i}",
                                   name=f"expT{i}_{n}")
                    nc.vector.tensor_mul(out=em[:],
                                         in0=expPair[i][:, :, 1, :],
                                         in1=expb_sb[:, 1, 2 * i:2 * i + 2, :])
                    expT1.append(em)
                if not IDENT_PB:
                    expT0 = []
                    for i in range(2):
                        em = expp.tile([P, 2, N], BF16, tag=f"expS{i}",
                                       name=f"expS{i}_{n}")
                        nc.vector.tensor_mul(
                            out=em[:], in0=expPair[i][:, :, 0, :],
                            in1=expb_sb[:, 0, 2 * i:2 * i + 2, :])
                        expT0.append(em)

                def rhs_for(t, h):
                    i, hi = h // 2, h % 2
                    if t == 1:
                        return expT1[i][:, hi, :]
                    if IDENT_PB:
                        return expPair[i][:, hi, 0, :]
                    return expT0[i][:, hi, :]

                # PE filler: next chunk's projection slice runs on the
                # tensor engine while ACT computes this row's exps.
                if filler is not None:
                    filler()

                # ---- sums (ones) + attn@V, col-packed, accum over t ----
                # separate banks: a start=True matmul clears has_written for
                # its whole bank row, so sums and o cannot share a bank.
                # su's second half is reused later for the W_O output (bank
                # bit-clears don't touch data outside the written region).
                su = acc.tile([P, 2, N], F32, tag="acc", name=f"su{n}")
                ov = acc.tile([P, 2, N], F32, tag="acc", name=f"ov{n}")
                for t in range(2):
                    for h in range(H):
                        nc.tensor.matmul(
                            su[32 * h:32 * h + 32, 0, :],
                            lhsT=ones_sb[:],
                            rhs=rhs_for(t, h),
                            start=(t == 0), stop=(t == 1),
                            tile_position=(0, 32 * h),
                            skip_group_check=True)
                    for h in range(H):
                        nc.tensor.matmul(
                            ov[32 * h:32 * h + 32, 0, :],
                            lhsT=vnat[:, q0 // P + t, 32 * h:32 * h + 32],
                            rhs=rhs_for(t, h),
                            start=(t == 0), stop=(t == 1),
                            tile_position=(0, 32 * h),
                            skip_group_check=True)

                # ---- normalize + gate:  go = ((1+tanh)*o) * (0.5/sum) ----
                inv = wrk.tile([P, N], F32, tag="inv", name=f"inv{n}")
                nc.vector.reciprocal_approx_fast(out=inv[:], in_=su[:, 0, :])
                go1 = wrk.tile([P, N], BF16, tag="go1", name=f"go1_{n}")
                nc.vector.scalar_tensor_tensor(
                    out=go1[:], in0=ghat[:, q0:q0 + N], scalar=1.0,
                    in1=ov[:, 0, :],
                    op0=mybir.AluOpType.add, op1=mybir.AluOpType.mult)
                go = wrk.tile([P, N], BF16, tag="go", name=f"go{n}")
                nc.vector.tensor_mul(out=go[:], in0=go1[:], in1=inv[:])

                # ---- W_O (w_o^T stationary): outT[cq, q] into su[:, 1, :] ----
                nc.tensor.matmul(su[:, 1, :],
                                 lhsT=wo_sb[:], rhs=go[:],
                                 start=True, stop=True,
                                 skip_group_check=True)
                pending_out.append((su, q0))

            # chunk 0's projections run up front; chunk c+1's are spread as
            # PE filler across chunk c's attention rows (8 halves / 8 rows)
            NPIECE = CROWS // PCH          # 4 proj pieces per chunk
            for ch in range(NPIECE):
                emit_proj_half(0, ch, 0)
                emit_proj_half(0, ch, 1)
            for c in range(NCHUNK):
                for i, n in enumerate(range(c * NPC, (c + 1) * NPC)):
                    filler = None
                    if c + 1 < NCHUNK:
                        filler = (lambda cc, cch, hh: lambda:
                                  emit_proj_half(cc, cch, hh))(
                                      c + 1, i // 2, i % 2)
                    emit_attn_row(n, filler=filler)
            flush_out()

    nc.compile()
    return nc


_CACHE: dict = {}


def _get_nc(use_mask: bool = False) -> bass.Bass:
    key = ("nc", use_mask)
    if key not in _CACHE:
        _CACHE[key] = build_nc(use_mask=use_mask)
    return _CACHE[key]


def make_in_maps(q_x, kv_x, mask_bias, pair_bias, w_q, w_k, w_v, w_g, w_o):
    qf = np.asarray(q_x, dtype=np.float32).reshape(NCORES, ROWS, CQ)
    kf = np.asarray(kv_x, dtype=np.float32).reshape(NCORES, ROWS, CQ)
    mb = np.asarray(mask_bias, dtype=np.float32).reshape(N, N)      # [n, k]
    pb = np.asarray(pair_bias, dtype=np.float32).reshape(H, N, N)   # [h, q, k]

    pbT = np.transpose(pb, (2, 0, 1))                 # [k, h, q]
    pb_dev = np.ascontiguousarray(pbT.reshape(2, P, H, N)).astype(NP_BF16)
    ident = np.eye(P, dtype=NP_BF16)

    wqt = np.ascontiguousarray((w_q / np.sqrt(C)).T).astype(NP_BF16)
    wkt = np.ascontiguousarray(np.asarray(w_k).T).astype(NP_BF16)
    wvt = np.ascontiguousarray(np.asarray(w_v).T).astype(NP_BF16)
    wgt = np.ascontiguousarray(np.asarray(w_g).T).astype(NP_BF16)
    wot = np.ascontiguousarray(np.asarray(w_o).T).astype(NP_BF16)   # [hc, cq]

    in_maps = []
    for c in range(NCORES):
        m = mb[c * NLOC:(c + 1) * NLOC]               # [nloc, k]
        mT = np.ascontiguousarray(
            np.transpose(m.reshape(NLOC, 2, P), (1, 2, 0))).astype(np.float32)
        in_maps.append({
            "q_xt": np.ascontiguousarray(qf[c].T).astype(NP_BF16),
            "kv_xt": np.ascontiguousarray(kf[c].T).astype(NP_BF16),
            "pbias": pb_dev,
            "ident": ident,
            "maskt": mT,
            "w_qt": wqt, "w_kt": wkt, "w_vt": wvt, "w_gt": wgt, "w_ot": wot,
        })
    return in_maps


def gather_out(res) -> np.ndarray:
    outs = []
    for c in range(NCORES):
        ot = np.asarray(res.results[c]["outT"], dtype=np.float32)  # [CQ, ROWS]
        outs.append(ot.T)                                          # [ROWS, CQ]
    return np.concatenate(outs, axis=0).reshape(B, N, N, CQ)


def kernel(q_x, kv_x, mask_bias, pair_bias, w_q, w_k, w_v, w_g, w_o):
    from concourse.bass_utils import run_bass_kernel_spmd

    nc = _get_nc(use_mask=bool(np.any(np.asarray(mask_bias))))
    in_maps = make_in_maps(q_x, kv_x, mask_bias, pair_bias,
                           w_q, w_k, w_v, w_g, w_o)
    res = run_bass_kernel_spmd(nc, in_maps, list(range(NCORES)))
    return gather_out(res)


# revision 31
# speedup vs baseline: 1.3538x; 1.0052x over previous
"""Trainium2 Bass kernel for AlphaFold-style pair attention (nn_Attention_90211493085692).

Reference computation (per batch b=1):
    q = (q_x @ w_q.T) / sqrt(C)         -> [N, Q, H, C]
    k = kv_x @ w_k.T ; v = kv_x @ w_v.T
    a = softmax(q @ k.T + mask_bias + pair_bias)   (softmax over k)
    o = (a @ v) * sigmoid(q_x @ w_g.T)
    out = o @ w_o.T

Sharding: outer pair dim N=256 split across 8 cores (32 rows each);
weights / pair_bias replicated, each core computes its slab independently.

Device-side strategy (per core, ROWS = 32*256 = 8192 token rows):
  - host pre-transposes q_x/kv_x slabs to [CQ=128, ROWS] bf16 so matmul
    contraction dims sit on SBUF partitions.
  - projections q^/k^ in transposed layout [hc, rows]; gate via tanh on ACT
    (sigmoid(x) = 0.5*(1+tanh(x/2)), the 0.5 baked into the softmax sums);
    v in natural layout [rows, hc].
  - scores per row n: scoresT[k, (h,q)] via 4-head row-packed K=32 matmuls.
    pair_bias handling is split to balance engines:
      t=0 (k in [0,128)):  identity-matmul copies pbT into PSUM first, score
                           matmuls accumulate on top -> exp(s+pb) directly.
      t=1 (k in [128,256)): exp(s) then one bf16 2x-mode DVE multiply by
                           precomputed exp(pb).
  - sums over k via col-packed ones(=2.0) matmul; attn@V col-packed with
    v-natural stationary -> o[hc, q]; gate+normalize: go = ((1+tanh)*o)*inv
    (STT from PSUM, then 2x TT); W_O with w_o^T stationary -> outT[cq, q];
    DVE copy PSUM->SBUF bf16 batched per 2 rows; DMA out; host transposes.
"""

import sys

sys.path.insert(0, "/opt/trn_rl_repo")
sys.path.insert(0, "/opt/pypackages")

from contextlib import ExitStack

import ml_dtypes
import numpy as np

import concourse.bass as bass
import concourse.bacc as bacc
import concourse.tile as tile
from concourse import mybir

H = 4
C = 32
CQ = 128
N = 256
B = 1
NCORES = 8
NLOC = N // NCORES          # 32 outer rows per core
ROWS = NLOC * N             # 8192 token rows per core
P = 128

F32 = mybir.dt.float32
BF16 = mybir.dt.bfloat16
NP_BF16 = ml_dtypes.bfloat16
AF = mybir.ActivationFunctionType


# pair_bias application for the k∈[0,128) half: True = identity-matmul into
# PSUM (PE), False = exp(pb) multiply on DVE (like the k∈[128,256) half).
IDENT_PB = True


def build_nc(use_mask: bool = False) -> bass.Bass:
    nc = bacc.Bacc()

    q_xt = nc.declare_dram_parameter("q_xt", [CQ, ROWS], BF16, isOutput=False)
    kv_xt = nc.declare_dram_parameter("kv_xt", [CQ, ROWS], BF16, isOutput=False)
    pbias = nc.declare_dram_parameter("pbias", [2, P, H, N], BF16, isOutput=False)
    ident = nc.declare_dram_parameter("ident", [P, P], BF16, isOutput=False)
    maskt = nc.declare_dram_parameter("maskt", [2, P, NLOC], F32, isOutput=False)
    w_qt = nc.declare_dram_parameter("w_qt", [CQ, P], BF16, isOutput=False)
    w_kt = nc.declare_dram_parameter("w_kt", [CQ, P], BF16, isOutput=False)
    w_vt = nc.declare_dram_parameter("w_vt", [CQ, P], BF16, isOutput=False)
    w_gt = nc.declare_dram_parameter("w_gt", [CQ, P], BF16, isOutput=False)
    w_ot = nc.declare_dram_parameter("w_ot", [P, CQ], BF16, isOutput=False)
    outT = nc.declare_dram_parameter("outT", [CQ, ROWS], BF16, isOutput=True)

    NCHUNK = 8
    CROWS = ROWS // NCHUNK         # 1024 rows per chunk
    NPC = NLOC // NCHUNK           # 4 attention rows per chunk

    with tile.TileContext(nc) as tc, ExitStack() as ctx:
        const = ctx.enter_context(tc.tile_pool(name="const", bufs=1))

        # small constants FIRST on both queues: the first projection matmul
        # needs the weights, so they must not queue behind 2MB of input
        wq_sb = const.tile([CQ, P], BF16)
        nc.sync.dma_start(out=wq_sb[:], in_=w_qt[:])
        wk_sb = const.tile([CQ, P], BF16)
        nc.sync.dma_start(out=wk_sb[:], in_=w_kt[:])
        wv_sb = const.tile([CQ, P], BF16)
        nc.sync.dma_start(out=wv_sb[:], in_=w_vt[:])
        wg_sb = const.tile([CQ, P], BF16)
        nc.sync.dma_start(out=wg_sb[:], in_=w_gt[:])
        wo_sb = const.tile([P, CQ], BF16)
        nc.sync.dma_start(out=wo_sb[:], in_=w_ot[:])
        if IDENT_PB:
            id_sb = const.tile([P, P], BF16)
            nc.scalar.dma_start(out=id_sb[:], in_=ident[:])

        mask_sb = const.tile([P, 2, NLOC], F32)
        nc.sync.dma_start(out=mask_sb[:], in_=maskt.rearrange("t p n -> p t n"))

        # pbT[p, t, h, q] = pair_bias[h, q, k = t*128+p]
        pb_sb = const.tile([P, 2, H, N], BF16)
        nc.scalar.dma_start(out=pb_sb[:], in_=pbias.rearrange("t p h q -> p t h q"))

        qx_sb = [const.tile([P, CROWS], BF16, name=f"qx_sb{c}")
                 for c in range(NCHUNK)]
        kx_sb = [const.tile([P, CROWS], BF16, name=f"kx_sb{c}")
                 for c in range(NCHUNK)]
        for c in range(NCHUNK):
            # alternate HWDGE issue engines so the two big loads stream in
            # parallel
            nc.sync.dma_start(out=qx_sb[c][:], in_=q_xt[:, c * CROWS:(c + 1) * CROWS])
            nc.scalar.dma_start(out=kx_sb[c][:], in_=kv_xt[:, c * CROWS:(c + 1) * CROWS])
        expb_sb = const.tile([P, 2, H, N], BF16)
        if IDENT_PB:
            nc.scalar.activation(out=expb_sb[:, 1], in_=pb_sb[:, 1], func=AF.Exp)
        else:
            nc.scalar.activation(out=expb_sb[:], in_=pb_sb[:], func=AF.Exp)

        # 2.0 (not 1.0): bakes the 0.5 of sigmoid(x)=0.5*(1+tanh(x/2)) into
        # the softmax denominator so inv = 0.5/sum
        ones_sb = const.tile([P, C], BF16)
        nc.vector.memset(ones_sb[:], 2.0)

        qhat = const.tile([P, ROWS], BF16)
        khat = const.tile([P, ROWS], BF16)
        ghat = const.tile([P, ROWS], BF16)
        vnat = const.tile([P, ROWS // P, P], BF16)

        PCH = 512                       # proj psum chunk (1 bank)
        with tc.tile_pool(name="scps", bufs=2, space="PSUM") as sp, \
             tc.tile_pool(name="accps", bufs=4, space="PSUM") as acc, \
             tc.tile_pool(name="work", bufs=2) as wrk, \
             tc.tile_pool(name="expp", bufs=3) as expp:

            def emit_proj_half(c, ch, half):
                """Projections for half of a 512-row slice of chunk c: PE
                filler emitted inside attention rows so the tensor engine
                has work while ACT runs the exps. half 0 = {q, g}, 1 = {k, v}."""
                base = c * CROWS
                sl = slice(ch * PCH, (ch + 1) * PCH)
                gsl = slice(base + ch * PCH, base + (ch + 1) * PCH)
                if half == 0:
                    ps_q = acc.tile([P, PCH], F32, tag="acc",
                                    name=f"ps_q{c}_{ch}")
                    nc.tensor.matmul(ps_q[:], lhsT=wq_sb[:],
                                     rhs=qx_sb[c][:, sl],
                                     start=True, stop=True)
                    nc.vector.tensor_copy(qhat[:, gsl], ps_q[:])

                    ps_g = acc.tile([P, PCH], F32, tag="acc",
                                    name=f"ps_g{c}_{ch}")
                    nc.tensor.matmul(ps_g[:], lhsT=wg_sb[:],
                                     rhs=qx_sb[c][:, sl],
                                     start=True, stop=True)
                    # tanh lives in the same ACT table set as exp, avoiding
                    # per-chunk table reloads; sigmoid(x)=0.5*(1+tanh(x/2))
                    nc.scalar.activation(out=ghat[:, gsl], in_=ps_g[:],
                                         func=AF.Tanh, scale=0.5)
                else:
                    ps_k = acc.tile([P, PCH], F32, tag="acc",
                                    name=f"ps_k{c}_{ch}")
                    nc.tensor.matmul(ps_k[:], lhsT=wk_sb[:],
                                     rhs=kx_sb[c][:, sl],
                                     start=True, stop=True)
                    nc.vector.tensor_copy(khat[:, gsl], ps_k[:])

                    ps_v = acc.tile([P, 4, P], F32, tag="acc",
                                    name=f"ps_v{c}_{ch}")
                    for j in range(4):
                        rt = ch * 4 + j
                        nc.tensor.matmul(
                            ps_v[:, j, :],
                            lhsT=kx_sb[c][:, rt * P:(rt + 1) * P],
                            rhs=wv_sb[:],
                            start=(j == 0), stop=(j == 3))
                    goff = base // P + ch * 4
                    nc.vector.tensor_copy(vnat[:, goff:goff + 4, :], ps_v[:])

            pending_out = []

            def flush_out():
                while pending_out:
                    psu, pq0 = pending_out.pop(0)
                    osb = wrk.tile([P, N], BF16, tag="osb", name=f"osb{pq0}")
                    # on ACT: the scalar engine has slack, the DVE doesn't
                    nc.scalar.copy(osb[:], psu[:, 1, :])
                    nc.scalar.dma_start(out=outT[:, pq0:pq0 + N], in_=osb[:])

            def emit_attn_row(n, filler=None):
                q0 = n * N
                # drain the previous row's W_O output first: it is ready, so
                # it never blocks the DVE queue, and it frees su's bank early
                flush_out()

                # ---- scores: two 2-bank tiles (head-pair each: bank = h%2,
                # FD-halves = t) so 4 row-packed matmuls drain into 4 distinct
                # banks (same-bank concurrent drains are a fatal PSUM
                # collision) AND bufs=2 lets row n+1's scores overlap row n's
                # exp — the stall that was keeping the PE clock-gated cold.
                scps = []
                for i in range(2):            # head pair i: heads 2i, 2i+1
                    scp = sp.tile([P, 2, 512], F32, tag="sc",
                                  name=f"sc_{n}_{i}")
                    if IDENT_PB:
                        for hi in range(2):
                            nc.tensor.matmul(
                                scp[:, hi, 0:N],
                                lhsT=id_sb[:],
                                rhs=pb_sb[:, 0, 2 * i + hi, :],
                                start=True, stop=False,
                                skip_group_check=True)
                    for t in range(2):
                        for hi in range(2):
                            h = 2 * i + hi
                            nc.tensor.matmul(
                                scp[:, hi, t * N:(t + 1) * N],
                                lhsT=khat[32 * h:32 * h + 32,
                                          q0 + t * P:q0 + (t + 1) * P],
                                rhs=qhat[32 * h:32 * h + 32, q0:q0 + N],
                                start=not (IDENT_PB and t == 0), stop=True,
                                tile_position=(32 * h, 0),
                                skip_group_check=True)
                    scps.append(scp)

                # exp per tile (FD=1024 contiguous); expPair[i][:, hi, t, :]
                expPair = []
                for i in range(2):
                    er = expp.tile([P, 2, 2, N], BF16, tag=f"expR{i}",
                                   name=f"expR{i}_{n}")
                    if use_mask:
                        # mask bias differs per k-half: two strided calls
                        for t in range(2):
                            nc.scalar.activation(
                                out=er[:, :, t, :],
                                in_=scps[i][:, :, t * N:(t + 1) * N],
                                func=AF.Exp, bias=mask_sb[:, t, n:n + 1])
                    else:
                        nc.scalar.activation(out=er[:], in_=scps[i][:],
                                             func=AF.Exp)
                    expPair.append(er)
                expT1 = []
                for i in range(2):
                    em = expp.tile([P, 2, N], BF16, tag=f"expT{i}",
                                   name=f"expT{i}_{n}")
                    nc.vector.tensor_mul(out=em[:],
                                         in0=expPair[i][:, :, 1, :],
                                         in1=expb_sb[:, 1, 2 * i:2 * i + 2, :])
                    expT1.append(em)
                if not IDENT_PB:
                    expT0 = []
                    for i in range(2):
                        em = expp.tile([P, 2, N], BF16, tag=f"expS{i}",
                                       name=f"expS{i}_{n}")
                        nc.vector.tensor_mul(
                            out=em[:], in0=expPair[i][:, :, 0, :],
                            in1=expb_sb[:, 0, 2 * i:2 * i + 2, :])
                        expT0.append(em)

                def rhs_for(t, h):
                    i, hi = h // 2, h % 2
                    if t == 1:
                        return expT1[i][:, hi, :]
                    if IDENT_PB:
                        return expPair[i][:, hi, 0, :]
                    return expT0[i][:, hi, :]

                # PE filler: next chunk's projection slice runs on the
                # tensor engine while ACT computes this row's exps.
                if filler is not None:
                    filler()

                # ---- sums (ones) + attn@V, col-packed, accum over t ----
                # separate banks: a start=True matmul clears has_written for
                # its whole bank row, so sums and o cannot share a bank.
                # su's second half is reused later for the W_O output (bank
                # bit-clears don't touch data outside the written region).
                su = acc.tile([P, 2, N], F32, tag="acc", name=f"su{n}")
                ov = acc.tile([P, 2, N], F32, tag="acc", name=f"ov{n}")
                for t in range(2):
                    for h in range(H):
                        nc.tensor.matmul(
                            su[32 * h:32 * h + 32, 0, :],
                            lhsT=ones_sb[:],
                            rhs=rhs_for(t, h),
                            start=(t == 0), stop=(t == 1),
                            tile_position=(0, 32 * h),
                            skip_group_check=True)
                    for h in range(H):
                        nc.tensor.matmul(
                            ov[32 * h:32 * h + 32, 0, :],
                            lhsT=vnat[:, q0 // P + t, 32 * h:32 * h + 32],
                            rhs=rhs_for(t, h),
                            start=(t == 0), stop=(t == 1),
                            tile_position=(0, 32 * h),
                            skip_group_check=True)

                # ---- normalize + gate:  go = ((1+tanh)*o) * (0.5/sum) ----
                inv = wrk.tile([P, N], F32, tag="inv", name=f"inv{n}")
                nc.vector.reciprocal_approx_fast(out=inv[:], in_=su[:, 0, :])
                go1 = wrk.tile([P, N], BF16, tag="go1", name=f"go1_{n}")
                nc.vector.scalar_tensor_tensor(
                    out=go1[:], in0=ghat[:, q0:q0 + N], scalar=1.0,
                    in1=ov[:, 0, :],
                    op0=mybir.AluOpType.add, op1=mybir.AluOpType.mult)
                go = wrk.tile([P, N], BF16, tag="go", name=f"go{n}")
                nc.vector.tensor_mul(out=go[:], in0=go1[:], in1=inv[:])

                # ---- W_O (w_o^T stationary): outT[cq, q] into su[:, 1, :] ----
                nc.tensor.matmul(su[:, 1, :],
                                 lhsT=wo_sb[:], rhs=go[:],
                                 start=True, stop=True,
                                 skip_group_check=True)
                pending_out.append((su, q0))

            # chunk 0's projections run up front; chunk c+1's are spread as
            # PE filler across chunk c's attention rows (8 halves / 8 rows)
            NPIECE = CROWS // PCH          # 4 proj pieces per chunk
            for ch in range(NPIECE):
                emit_proj_half(0, ch, 0)
                emit_proj_half(0, ch, 1)
            for c in range(NCHUNK):
                for i, n in enumerate(range(c * NPC, (c + 1) * NPC)):
                    filler = None
                    if c + 1 < NCHUNK:
                        filler = (lambda cc, cch, hh: lambda:
                                  emit_proj_half(cc, cch, hh))(
                                      c + 1, i // 2, i % 2)
                    emit_attn_row(n, filler=filler)
            flush_out()

    nc.compile()
    return nc


_CACHE: dict = {}


def _get_nc(use_mask: bool = False) -> bass.Bass:
    key = ("nc", use_mask)
    if key not in _CACHE:
        _CACHE[key] = build_nc(use_mask=use_mask)
    return _CACHE[key]


def make_in_maps(q_x, kv_x, mask_bias, pair_bias, w_q, w_k, w_v, w_g, w_o):
    qf = np.asarray(q_x, dtype=np.float32).reshape(NCORES, ROWS, CQ)
    kf = np.asarray(kv_x, dtype=np.float32).reshape(NCORES, ROWS, CQ)
    mb = np.asarray(mask_bias, dtype=np.float32).reshape(N, N)      # [n, k]
    pb = np.asarray(pair_bias, dtype=np.float32).reshape(H, N, N)   # [h, q, k]

    pbT = np.transpose(pb, (2, 0, 1))                 # [k, h, q]
    pb_dev = np.ascontiguousarray(pbT.reshape(2, P, H, N)).astype(NP_BF16)
    ident = np.eye(P, dtype=NP_BF16)

    wqt = np.ascontiguousarray((w_q / np.sqrt(C)).T).astype(NP_BF16)
    wkt = np.ascontiguousarray(np.asarray(w_k).T).astype(NP_BF16)
    wvt = np.ascontiguousarray(np.asarray(w_v).T).astype(NP_BF16)
    wgt = np.ascontiguousarray(np.asarray(w_g).T).astype(NP_BF16)
    wot = np.ascontiguousarray(np.asarray(w_o).T).astype(NP_BF16)   # [hc, cq]

    in_maps = []
    for c in range(NCORES):
        m = mb[c * NLOC:(c + 1) * NLOC]               # [nloc, k]
        mT = np.ascontiguousarray(
            np.transpose(m.reshape(NLOC, 2, P), (1, 2, 0))).astype(np.float32)
        in_maps.append({
            "q_xt": np.ascontiguousarray(qf[c].T).astype(NP_BF16),
            "kv_xt": np.ascontiguousarray(kf[c].T).astype(NP_BF16),
            "pbias": pb_dev,
            "ident": ident,
            "maskt": mT,
            "w_qt": wqt, "w_kt": wkt, "w_vt": wvt, "w_gt": wgt, "w_ot": wot,
        })
    return in_maps


def gather_out(res) -> np.ndarray:
    outs = []
    for c in range(NCORES):
        ot = np.asarray(res.results[c]["outT"], dtype=np.float32)  # [CQ, ROWS]
        outs.append(ot.T)                                          # [ROWS, CQ]
    return np.concatenate(outs, axis=0).reshape(B, N, N, CQ)


def kernel(q_x, kv_x, mask_bias, pair_bias, w_q, w_k, w_v, w_g, w_o):
    from concourse.bass_utils import run_bass_kernel_spmd

    nc = _get_nc(use_mask=bool(np.any(np.asarray(mask_bias))))
    in_maps = make_in_maps(q_x, kv_x, mask_bias, pair_bias,
                           w_q, w_k, w_v, w_g, w_o)
    res = run_bass_kernel_spmd(nc, in_maps, list(range(NCORES)))
    return gather_out(res)
